# revision 70
# baseline (speedup 1.0000x reference)
"""Causal multi-head attention on 8 Trainium2 NeuronCores.

Sharding: data-parallel over batch (B=2) x tensor-parallel over heads
(16 heads -> 4 groups of 4). Core c handles batch c//4, head group c%4.
Each core computes q/k/v projections for its 4 heads, causal flash
attention, and a partial output projection (row slice of Wo); the host
sums the 4 partials per batch element.

All transposes happen on the HOST, and every input is pre-arranged in
its exact SBUF tile layout so each DMA lands as 128 contiguous 4-8KB
per-partition runs (the DGE issues ~170 descriptors/us, so descriptor
COUNT — not HBM bandwidth — gates the load when runs are 512B-1KB).
All bulk inputs stream on one HW-DGE queue in priority order (wq, x
sc0, wk, wv, x sc1, wo, x sc2-3); the output y^T is staged per window
and leaves in 2-4 wide DMAs per window for the same reason.

Matmuls run in bf16 (fp32 PSUM accumulation). The softmax row-sum is
fused into the o^T = [v|1s]^T P^T matmul via an appended ones column;
normalization (broadcast rowsum via K=1 matmul, fast-approx
reciprocal, divide) stays in fp32.

Engine assignment: ACT runs the exp stream and nothing else mid-window
(it co-paces the attention slots with the PE); DVE takes all
psum->sbuf copies + reciprocal; GpSimd takes the causal-mask
multiplies and v-scatters.  A ~100-matmul warm-up spin on a zeroed
tile runs during the initial DMA wait so the PE HAM clock-gate is at
8/8 (2.4 GHz) when real work starts.

Scheduling: per (q-chunk, head-pair) only the row-tiled S^T = k q^T
pair and the masked exp are emitted in the main loop; everything else
(projection chunks, AV matmuls, epilogues, output projections) drains
from a work queue between QK slots, paced in estimated PE-ns so fuel
neither bunches into ACT-starving bursts nor runs dry before the last
window (either one lets the HAM re-throttle to half clock).  Six
output-projection units of the second-to-last window are reserved and
replayed at the very end, where the PE would otherwise idle ~3us
behind the final epilogue's DVE chain.
"""

import numpy as np
import ml_dtypes

import concourse.bacc as bacc
import concourse.bass as bass
import concourse.tile as tile
from concourse import bass_utils, mybir

B, S, D, H = 2, 2048, 1024, 16
DK = 64
NH = 4                 # heads per core
E = NH * DK            # 256: per-core head-dim slice
SCALE = 1.0 / 8.0      # 1/sqrt(DK)

F32 = mybir.dt.float32
F32R = mybir.dt.float32r
BF16 = mybir.dt.bfloat16

QC = 512               # q-chunk (columns per attention tile)
NQC = S // QC          # 4
NKB = S // 128         # 16 k-blocks


def _emit(tc, nc, xT_d, wqT_d, wkT_d, wvT_d, woT_d, yT_d, mask_d, ones_d):
    const = tc.alloc_tile_pool(name="const", bufs=1)
    perm = tc.alloc_tile_pool(name="perm", bufs=1)
    p01 = tc.alloc_tile_pool(name="p01", bufs=1)

    mask = const.tile([128, 128], BF16)
    ones_f32 = const.tile([128, 64], F32)
    ones128 = const.tile([128, 64], BF16)
    warm_w = const.tile([64, 64], BF16)   # garbage-free spin operand

    woT = perm.tile([128, 2, D], BF16)   # woT[p, ec, o] = wo[o, ec*128+p]
    qT = perm.tile([128, 2, S], BF16)    # qT[p, ec, s] = q[s, ec*128+p]
    kT = perm.tile([128, 2, S], BF16)
    v_sb = perm.tile([128, NKB, NH, DK + 1], BF16)  # [.., 64] = ones column

    xT = p01.tile([128, 8, S], BF16)     # xT[p, dc, s] = x[s, dc*128+p]
    wqT = p01.tile([128, 8, E], BF16)    # wqT[p, dc, e] = wq[e, dc*128+p]
    wkT = p01.tile([128, 8, E], BF16)
    wvT = p01.tile([128, 8, E], BF16)

    # the PE warm-up spin (emitted below) only depends on this memset:
    # it MUST be the first gpsimd instruction, ahead of the slow SWDGE
    # descriptor generation for the gpsimd-queue DMAs
    nc.gpsimd.memset(warm_w, 0.0)

    # ALL bulk inputs go on ONE queue (sync, HW DGE) in strict priority
    # order.  The host pre-arranges every tensor in its exact SBUF tile
    # layout, so each DMA is 128 contiguous 4-8KB runs (the DGE issue
    # rate of ~170 descriptors/us, not HBM bandwidth, was the gate when
    # the runs were 512B-1KB per-partition lines)
    def x_chunk(sc, half=None):
        dcs = slice(0, 8) if half is None else slice(half * 4, half * 4 + 4)
        n = 4096 if half is None else 2048
        nc.sync.dma_start(
            out=xT[:, dcs, sc * 512:(sc + 1) * 512],
            in_=bass.AP(
                tensor=xT_d.tensor,
                offset=sc * 128 * 4096 + (0 if half is None else half * 2048),
                ap=[[4096, 128], [1, n]],
            ),
        )

    # wq on the scalar queue (HW DGE, otherwise idle) so it lands in
    # parallel with xT sc0 from the sync queue; both in dc-halves so the
    # first projection chain starts on dc0-3 while dc4-7 is in flight
    nc.scalar.dma_start(out=wqT[:, 0:4, :], in_=wqT_d[:, 0:4, :])
    nc.scalar.dma_start(out=wqT[:, 4:8, :], in_=wqT_d[:, 4:8, :])
    x_chunk(0, 0)
    x_chunk(0, 1)
    nc.sync.dma_start(out=wkT[:, 0:4, :], in_=wkT_d[:, 0:4, :])
    nc.sync.dma_start(out=wkT[:, 4:8, :], in_=wkT_d[:, 4:8, :])
    nc.sync.dma_start(out=wvT, in_=wvT_d)
    x_chunk(1)
    nc.sync.dma_start(out=woT, in_=woT_d)
    x_chunk(2)
    x_chunk(3)
    nc.gpsimd.dma_start(out=mask, in_=mask_d)
    nc.gpsimd.dma_start(out=ones_f32, in_=ones_d)
    # ones row for the rowsum broadcast (row 64 used as lhsT)
    nc.vector.tensor_copy(ones128, ones_f32)

    def copy(dst, src):
        # ACT is reserved for the exp stream (it paces the attention
        # windows): all psum->sbuf copies go to the DVE
        nc.vector.tensor_copy(dst, src)

    work = tc.alloc_tile_pool(name="work", bufs=3)
    small = tc.alloc_tile_pool(name="small", bufs=2)

    # ---- phases 1-3 fused: the attention windows are exp(ACT)-paced, so
    # the q/k/v projections (pure PE work) drain INTO the windows as
    # background fuel; window qc only needs proj chunks sc <= qc ----
    with tc.tile_pool(name="ps01", bufs=1, space="PSUM") as ps01, \
         tc.tile_pool(name="psS", bufs=1, space="PSUM") as ps_S, \
         tc.tile_pool(name="psO", bufs=1, space="PSUM") as ps_o:
        ps_y = ps01

        # HAM warm-up: the PE clock-gate defaults to 4/8 (1.2 GHz) and
        # only opens to 8/8 after ~3.4us of sustained activity.  The
        # first ~4us of the kernel is DMA wait anyway, so spin small
        # matmuls on a zeroed tile: by the time wq/xT land, the PE runs
        # the real projections at full clock instead of half.
        warm_ps = ps01.tile([64, 64], F32, tag="y", bufs=2, name="warmps")
        for _ in range(72):
            nc.tensor.matmul(warm_ps, lhsT=warm_w, rhs=warm_w, start=True, stop=True)

        # ones column of v (written once; strided 3D AP)
        ones_ap = bass.AP(
            tensor=v_sb.tensor,
            offset=v_sb.offset + DK,
            ap=[v_sb.ap[0], [NH * (DK + 1), NKB], [DK + 1, NH]],
        )
        src64 = bass.AP(
            tensor=ones_f32.tensor, offset=ones_f32.offset,
            ap=[ones_f32.ap[0], [4, NKB], [1, NH]],
        )
        nc.vector.tensor_copy(ones_ap, src64)

        # touch exp once so the ~2.7us ACT table load happens during the
        # projection warm-up instead of stalling the first QK window
        warm = const.tile([1, 4], F32)
        nc.scalar.activation(
            warm, ones_f32[0:1, 0:4], mybir.ActivationFunctionType.Exp
        )

        def make_proj(w_t, outT, ec, sc):
            def u():
                ps = ps01.tile([128, 512], F32, tag="y", bufs=2, name="psp")
                for dc in range(8):
                    nc.tensor.matmul(
                        ps,
                        lhsT=w_t[:, dc, ec * 128:(ec + 1) * 128],
                        rhs=xT[:, dc, sc * 512:(sc + 1) * 512],
                        start=(dc == 0),
                        stop=(dc == 7),
                    )
                copy(outT[:, ec, sc * 512:(sc + 1) * 512], ps)
            return u

        # only the (sc0, ec0) q/k chunks are needed before window 0's
        # hp0 chain: emit those inline, everything else becomes fuel
        # (the ec1 chunks are only read by each window's hp1 chain)
        projq = []
        for sc in range(4):
            for w_t, outT in [(wqT, qT), (wkT, kT)]:
                for ec in range(2):
                    if sc == 0 and ec == 0:
                        make_proj(w_t, outT, ec, sc)()
                    else:
                        projq.append((sc, ec, make_proj(w_t, outT, ec, sc)))

        # warm-clock PE-ns cost estimates per unit kind, used to pace the
        # drain in time units (a unit-count budget lets cheap units bunch
        # and starve the ACT exp stream at window boundaries)
        PROJ_NS = 1730     # 8 matmuls N=512
        VPROJ_NS = 880     # 8 matmuls N=256
        AV_NS = 500        # matmul pair N<=512
        OUTPROJ_NS = 430   # 2 matmuls N=512
        EPI_NS = 300       # bc matmul pair

        # (cost_ns, min_slot, closure): a unit may only drain once
        # slot_i >= min_slot (keeps AV >= 3 QK slots behind its exp, and
        # defers vproj(kb) to the first window that consumes it)
        workq = []
        slot_i = [0]
        fuel_ns = [len(projq) * PROJ_NS]

        def make_vproj(sblk):
            def u():
                ps = ps01.tile([128, E], F32, tag="y", bufs=2, name="psv")
                for dc in range(8):
                    nc.tensor.matmul(
                        ps,
                        lhsT=xT[:, dc, sblk * 128:(sblk + 1) * 128],
                        rhs=wvT[:, dc, :],
                        start=(dc == 0),
                        stop=(dc == 7),
                    )
                # scatter 4 heads into [.., l, 0:64]
                sap = bass.AP(
                    tensor=ps.tensor, offset=ps.offset,
                    ap=[ps.ap[0], [DK, NH], [1, DK]],
                )
                nc.vector.tensor_copy(v_sb[:, sblk, :, 0:DK], sap)
            return u

        # attention windows (q-column ranges)
        WINS = [(0, 512), (512, 512), (1024, 512), (1536, 512)]
        _starts = []
        _acc = 0
        for (_q0, _w) in WINS:
            _starts.append(_acc)
            _acc += 2 * ((_q0 + _w) // 128)
        TOTAL_SLOTS = _acc                                           # 108

        def _vp_start(kb):
            for (s, (_q0, _w)) in zip(_starts, WINS):
                if kb * 128 < _q0 + _w:
                    return s
            return 0

        for sblk in range(NKB):
            workq.append((VPROJ_NS, _vp_start(sblk), make_vproj(sblk)))
            fuel_ns[0] += VPROJ_NS

        # drain pacing: spread the queued PE work evenly (in estimated
        # ns) over the remaining QK slots, so fuel neither bunches into
        # ACT-starving bursts nor runs dry before the last window; scan
        # past not-yet-eligible units (safe: relative order of dependent
        # units is preserved by min_slot construction)
        def drain_some():
            slots_left = max(1, TOTAL_SLOTS - slot_i[0])
            target = min(3400, fuel_ns[0] // slots_left)
            spent = 0
            j = 0
            while j < len(workq) and spent < target:
                if workq[j][1] <= slot_i[0]:
                    c, _, u = workq.pop(j)
                    u()
                    fuel_ns[0] -= c
                    spent += c
                else:
                    j += 1
            # proj chunk sc's xT arrives ~2-3us per chunk after kernel
            # start: don't pop it as fuel before its data can be there
            PROJ_GATE = {0: 0, 1: 0, 2: 2, 3: 4}
            if spent < target and projq and slot_i[0] >= PROJ_GATE[projq[0][0]]:
                _, _, u = projq.pop(0)
                u()
                fuel_ns[0] -= PROJ_NS

        def make_av(po_box, pts, kb, hp, kmax, width, lw=False):
            last = kb == kmax - 1

            def av():
                if po_box[0] is None:
                    po_box[0] = (
                        ps_o.tile([DK + 1, QC], F32, tag="o", bufs=2, name="poA"),
                        ps_o.tile([DK + 1, QC], F32, tag="o", bufs=2, name="poB"),
                    )
                poA, poB = po_box[0]
                pT, cs = pts[kb]
                for hi, po in ((0, poA), (1, poB)):
                    nc.tensor.matmul(
                        po[:, cs:width],
                        lhsT=v_sb[:, kb, 2 * hp + hi, :],
                        rhs=pT[:, hi, cs:width],
                        start=(kb == 0),
                        stop=last,
                    )
                if last:
                    # stage the epilogue inputs: both heads' o^T copied
                    # out to SBUF immediately so the poA/poB PSUM banks
                    # free HERE (the next head-pair's first AVs wait on
                    # this ring — leaving poA to be read by the epilogue
                    # mul kept the bank busy ~2.5us longer).  rsA goes on
                    # ACT (idle between exps, tiny) so bcA isn't queued
                    # behind the oB copy on the DVE.
                    rsA = small.tile([1, QC], BF16, tag="rsA", bufs=2)
                    oA_sb = small.tile([DK, QC], BF16, tag="oAsb", bufs=2)
                    oB_sb = small.tile([DK + 1, QC], BF16, tag="osb", bufs=4)
                    nc.scalar.copy(rsA[:, 0:width], poA[DK:DK + 1, 0:width])
                    nc.vector.tensor_copy(oA_sb[:, 0:width], poA[0:DK, 0:width])
                    nc.vector.tensor_copy(oB_sb[:, 0:width], poB[:, 0:width])
                    oB_hi = small.tile([128, QC], BF16, tag="oBhi", bufs=2)
                    nc.gpsimd.dma_start(
                        out=oB_hi[64:128, 0:width], in_=oB_sb[0:DK, 0:width]
                    )
                    po_box[1] = (rsA, oA_sb, oB_sb, oB_hi)
            return av

        def make_epilogue(po_box, oT, hp, width):
            def epi():
                rsA, oA_sb, oB_sb, oB_hi = po_box[1]
                # both rowsums broadcast into one psum bank (head A to
                # partitions 0-63, head B to 64-127) so a single
                # 128-partition reciprocal feeds both divides; head B's
                # o^T was already shifted to partitions 64-127 right
                # after its last AV, off this critical path
                ps_bc = ps_y.tile([128, QC], F32, tag="y", bufs=2, name="psbc")
                nc.tensor.matmul(
                    ps_bc[0:64, 0:width],
                    lhsT=ones128[0:1, :],
                    rhs=rsA[:, 0:width],
                    start=True,
                    stop=True,
                )
                nc.tensor.matmul(
                    ps_bc[64:128, 0:width],
                    lhsT=ones128[64:65, :],
                    rhs=oB_sb[DK:DK + 1, 0:width],
                    start=True,
                    stop=True,
                )
                # split the reciprocal and run the two normalize
                # multiplies on different engines (DVE + GpSimd) so the
                # epilogue critical path is one recip + one mul, not a
                # serial recip->mulA->mulB chain on the DVE
                rec = small.tile([128, QC], F32, tag="rec", bufs=2)
                nc.vector.reciprocal_approx_fast(rec[:, 0:width], ps_bc[:, 0:width])
                nc.vector.tensor_mul(
                    oT[0:64, hp, 0:width], oA_sb[:, 0:width], rec[0:64, 0:width]
                )
                nc.vector.tensor_mul(
                    oT[64:128, hp, 0:width],
                    oB_hi[64:128, 0:width],
                    rec[64:128, 0:width],
                )
            return epi

        def make_out_proj(wi, q0, width, is_last, oT):
            # one window-wide staging tile and TWO output DMAs (4KB
            # per-partition runs) instead of 8 per-dc DMAs with 1KB
            # lines: the output path was DGE-issue-bound (~170
            # descriptors/us), which dominated the kernel tail
            y_all = work.tile([128, 8, QC], BF16, tag="ysb", bufs=2)
            box = {}
            units = []
            for dc in range(8):
                def u(dc=dc, oT=oT):
                    if is_last:
                        # final window: QK is done, so the score banks are
                        # free — alternate units between the S ring and
                        # the y ring so FOUR accumulations are in flight
                        # and the tail isn't copy-throttled
                        if dc % 2 == 0:
                            psyt = ps_S.tile([128, 2, 512], F32, tag="S",
                                             bufs=2, name="psyt")
                            psy = psyt[:, 0, 0:width]
                        else:
                            psyf = ps_y.tile([128, QC], F32, tag="y", bufs=2,
                                             name="psy")
                            psy = psyf[:, 0:width]
                    else:
                        psyf = ps_y.tile([128, QC], F32, tag="y", bufs=2,
                                         name="psy")
                        psy = psyf[:, 0:width]
                    for ec in range(2):
                        nc.tensor.matmul(
                            psy,
                            lhsT=woT[:, ec, dc * 128:(dc + 1) * 128],
                            rhs=oT[:, ec, 0:width],
                            start=(ec == 0),
                            stop=(ec == 1),
                        )
                    # parity split: consecutive units' copies alternate
                    # engines so the tail drains two copies at a time
                    if dc % 2 == 0:
                        nc.vector.tensor_copy(y_all[:, dc, 0:width], psy)
                    else:
                        nc.scalar.copy(y_all[:, dc, 0:width], psy)
                    # finer DMA granularity in the last window so the
                    # final (chain-ending) transfer is 256KB, not 512KB
                    step = 2 if is_last else 4
                    if dc % step == step - 1:
                        h0 = dc - step + 1
                        nc.sync.dma_start(
                            out=bass.AP(
                                tensor=yT_d.tensor,
                                offset=wi * 128 * 4096 + h0 * 512,
                                ap=[[4096, 128], [512, step], [1, 512]],
                            ),
                            in_=y_all[:, h0:dc + 1, 0:width],
                        )
                units.append(u)
            return units

        for wi, (q0, width) in enumerate(WINS):
            sc_need = (q0 + width - 1) // 512
            oT = work.tile([128, 2, QC], BF16, tag="oT", bufs=2)
            kmax = (q0 + width) // 128
            is_last = wi == len(WINS) - 1
            for hp in range(2):
                # this hp chain reads q/k chunks (sc <= sc_need, ec <= hp):
                # force any not yet drained (usually already gone as fuel)
                ii = 0
                while ii < len(projq):
                    sc_, ec_, fn_ = projq[ii]
                    if sc_ <= sc_need and ec_ <= hp:
                        projq.pop(ii)
                        fn_()
                        fuel_ns[0] -= PROJ_NS
                    else:
                        ii += 1
                pts = {}
                po_box = [None, None]
                for kb in range(kmax):
                    # drain BEFORE the QK pair: the drained AV/fuel MMs are
                    # long (213-432ns), so both their LDWEIGHTS and the QK
                    # pair's hide under preceding streams.  (With QK first,
                    # the AV's LDW lands right after the 2nd row-tiled QK
                    # matmul — which finishes ~3ns after the 1st — and is
                    # fully exposed, ~118ns per slot.)
                    drain_some()
                    # S^T = k q^T, 2-head row-tiled pair, causally narrowed
                    cs = max(0, kb * 128 - q0)
                    psS = ps_S.tile([128, 2, 512], F32, tag="S", bufs=2)
                    for hi in range(2):
                        nc.tensor.matmul(
                            psS[:, hi, cs:width],
                            lhsT=kT[hi * 64:(hi + 1) * 64, hp,
                                    kb * 128:(kb + 1) * 128],
                            rhs=qT[hi * 64:(hi + 1) * 64, hp,
                                   q0 + cs:q0 + width],
                            start=True,
                            stop=True,
                        )
                    pT = work.tile([128, 2, 512], BF16, tag="pT", bufs=32)
                    pts[kb] = (pT, cs)
                    nc.scalar.activation(
                        pT[:, :, cs:width],
                        psS[:, :, cs:width],
                        mybir.ActivationFunctionType.Exp,
                        scale=SCALE,
                    )
                    if kb * 128 >= q0:  # diagonal band: zero upper triangle
                        # 0/1 multiply AFTER exp, on the deeply-buffered pT;
                        # one op for both heads via a stride-0 middle dim.
                        # On GpSimd: the DVE is loaded with psum->sbuf
                        # copies and ACT must stay exp-only
                        mask2 = bass.AP(
                            tensor=mask.tensor, offset=mask.offset,
                            ap=[mask.ap[0], [0, 2], mask.ap[1]],
                        )
                        nc.gpsimd.tensor_mul(
                            pT[:, :, cs:cs + 128],
                            pT[:, :, cs:cs + 128],
                            mask2,
                        )
                    # AV for this kb becomes available 3 QK slots later
                    workq.append(
                        (AV_NS, slot_i[0] + 3,
                         make_av(po_box, pts, kb, hp, kmax, width, lw=is_last))
                    )
                    fuel_ns[0] += AV_NS
                    slot_i[0] += 1
                # epi min matches its last AV's eligibility; outproj one
                # later (list order breaks the tie in favor of epi)
                workq.append((EPI_NS, slot_i[0] + 2, make_epilogue(po_box, oT, hp, width)))
                fuel_ns[0] += EPI_NS
                if hp == 1:
                    units = make_out_proj(wi, q0, width, is_last, oT)
                    # reserve half of the second-to-last window's outproj
                    # as tail fuel: at the very end the PE would otherwise
                    # idle ~2.7us behind the exp->AV->epilogue chain of the
                    # final window (these units are independent of it)
                    reserve = 6 if wi == len(WINS) - 2 else 0
                    for ui, u in enumerate(units):
                        ms = 10 ** 9 if ui >= len(units) - reserve else slot_i[0] + 3
                        workq.append((OUTPROJ_NS, ms, u))
                    fuel_ns[0] += OUTPROJ_NS * len(units)
        # final flush.  The tail dependency chain is last-exp -> last-AVs
        # -> epilogue copies -> bc/recip/muls -> final outproj; the
        # reserved units (min_slot sentinel) are independent of all of
        # it, so emit them BETWEEN the AVs and the epilogue where the PE
        # would otherwise idle ~3us behind the DVE/ACT epilogue chain.
        reserved = [e for e in workq if e[1] >= 10 ** 8]
        rest = [e for e in workq if e[1] < 10 ** 8]
        last_epi = max(
            (i for i, e in enumerate(rest) if e[0] == EPI_NS), default=0
        )
        # 2 reserved units cover the epilogue-staging copies, the rest
        # cover the bc->recip->mul DVE chain (~2.5us) that the final
        # outproj waits on
        for _, _, u in rest[:last_epi + 1]:
            u()
        for _, _, u in reserved:
            u()
        for _, _, u in rest[last_epi + 1:]:
            u()

    for p in [small, work, p01, perm, const]:
        p.release()


_CACHE = {}


def _build():
    if "nc" in _CACHE:
        return _CACHE["nc"]
    nc = bacc.Bacc("TRN2", target_bir_lowering=False, debug=False, num_devices=8)
    # all inputs pre-arranged host-side into their SBUF tile layouts so
    # every DMA lands as 128 contiguous 4-8KB per-partition runs
    xT_d = nc.dram_tensor("xT", [4, 128, 4096], BF16, kind="ExternalInput").ap()
    wqT_d = nc.dram_tensor("wqT", [128, 8, E], BF16, kind="ExternalInput").ap()
    wkT_d = nc.dram_tensor("wkT", [128, 8, E], BF16, kind="ExternalInput").ap()
    wvT_d = nc.dram_tensor("wvT", [128, 8, E], BF16, kind="ExternalInput").ap()
    woT_d = nc.dram_tensor("woT", [128, 2, D], BF16, kind="ExternalInput").ap()
    yT_d = nc.dram_tensor("yT", [4, 128, 4096], BF16, kind="ExternalOutput").ap()
    mask_d = nc.dram_tensor("maskc", [128, 128], BF16, kind="ExternalInput").ap()
    ones_d = nc.dram_tensor("onesc", [128, 64], F32, kind="ExternalInput").ap()
    with tile.TileContext(nc) as tc:
        _emit(tc, nc, xT_d, wqT_d, wkT_d, wvT_d, woT_d, yT_d, mask_d, ones_d)
    nc.compile()
    _CACHE["nc"] = nc
    return nc


_r = np.arange(128)
_MASK = np.where(_r[:, None] <= _r[None, :], 1.0, 0.0).astype(ml_dtypes.bfloat16)
_ONES = np.ones((128, 64), dtype=np.float32)

LAST_RESULT = None


def kernel(x, wq, wk, wv, wo):
    global LAST_RESULT
    nc = _build()
    bf = ml_dtypes.bfloat16
    x = np.asarray(x, dtype=np.float32)
    wq = np.asarray(wq, dtype=np.float32)
    wk = np.asarray(wk, dtype=np.float32)
    wv = np.asarray(wv, dtype=np.float32)
    wo = np.asarray(wo, dtype=np.float32)

    def wlay(w2d):  # [256 out, 1024 in] -> [p, dc, e] = w2d[e, dc*128+p]
        return np.ascontiguousarray(
            w2d.T.reshape(8, 128, 256).transpose(1, 0, 2).astype(bf))

    in_maps = []
    for c in range(8):
        b, g = c // 4, c % 4
        rows = slice(g * E, (g + 1) * E)
        # xT dram [sc, p, dc*512+c] = x[sc*512+c, dc*128+p]
        xr = x[b].reshape(4, 512, 8, 128).transpose(0, 3, 2, 1).reshape(4, 128, 4096)
        # woT dram [p, ec, o] = wo[o, ec*128+p (within rows)]
        wor = wo[:, rows].T.reshape(2, 128, 1024).transpose(1, 0, 2)
        in_maps.append({
            "xT": np.ascontiguousarray(xr.astype(bf)),
            "wqT": wlay(wq[rows]),
            "wkT": wlay(wk[rows]),
            "wvT": wlay(wv[rows]),
            "woT": np.ascontiguousarray(wor.astype(bf)),
            "maskc": _MASK,
            "onesc": _ONES,
        })

    res = bass_utils.run_bass_kernel_spmd(nc, in_maps, core_ids=list(range(8)))
    LAST_RESULT = res

    y = np.empty((B, S, D), dtype=np.float32)
    for b in range(B):
        acc = res.results[4 * b]["yT"].astype(np.float32)
        for g in range(1, 4):
            acc += res.results[4 * b + g]["yT"].astype(np.float32)
        # yT dram [wi, p, dc*512+c] = y[wi*512+c, dc*128+p]
        y[b] = acc.reshape(4, 128, 8, 512).transpose(0, 3, 2, 1).reshape(S, D)
    return y



# revision 72
# speedup vs baseline: 1.0152x; 1.0152x over previous
"""Causal multi-head attention on 8 Trainium2 NeuronCores.

Sharding: data-parallel over batch (B=2) x tensor-parallel over heads
(16 heads -> 4 groups of 4). Core c handles batch c//4, head group c%4.
Each core computes q/k/v projections for its 4 heads, causal flash
attention, and a partial output projection (row slice of Wo); the host
sums the 4 partials per batch element.

All transposes happen on the HOST, and every input is pre-arranged in
its exact SBUF tile layout so each DMA lands as 128 contiguous 4-8KB
per-partition runs (the DGE issues ~170 descriptors/us, so descriptor
COUNT — not HBM bandwidth — gates the load when runs are 512B-1KB).
All bulk inputs stream on one HW-DGE queue in priority order (wq, x
sc0, wk, wv, x sc1, wo, x sc2-3); the output y^T is staged per window
and leaves in 2-4 wide DMAs per window for the same reason.

Matmuls run in bf16 (fp32 PSUM accumulation). The softmax row-sum is
fused into the o^T = [v|1s]^T P^T matmul via an appended ones column;
normalization (broadcast rowsum via K=1 matmul, fast-approx
reciprocal, divide) stays in fp32.

Engine assignment: ACT runs the exp stream and nothing else mid-window
(it co-paces the attention slots with the PE); DVE takes all
psum->sbuf copies + reciprocal; GpSimd takes the causal-mask
multiplies and v-scatters.  A ~100-matmul warm-up spin on a zeroed
tile runs during the initial DMA wait so the PE HAM clock-gate is at
8/8 (2.4 GHz) when real work starts.

Scheduling: per (q-chunk, head-pair) only the row-tiled S^T = k q^T
pair and the masked exp are emitted in the main loop; everything else
(projection chunks, AV matmuls, epilogues, output projections) drains
from a work queue between QK slots, paced in estimated PE-ns so fuel
neither bunches into ACT-starving bursts nor runs dry before the last
window (either one lets the HAM re-throttle to half clock).  Six
output-projection units of the second-to-last window are reserved and
replayed at the very end, where the PE would otherwise idle ~3us
behind the final epilogue's DVE chain.
"""

import numpy as np
import ml_dtypes

import concourse.bacc as bacc
import concourse.bass as bass
import concourse.tile as tile
from concourse import bass_utils, mybir

B, S, D, H = 2, 2048, 1024, 16
DK = 64
NH = 4                 # heads per core
E = NH * DK            # 256: per-core head-dim slice
SCALE = 1.0 / 8.0      # 1/sqrt(DK)

F32 = mybir.dt.float32
F32R = mybir.dt.float32r
BF16 = mybir.dt.bfloat16

QC = 512               # q-chunk (columns per attention tile)
NQC = S // QC          # 4
NKB = S // 128         # 16 k-blocks


def _emit(tc, nc, xT_d, wqT_d, wkT_d, wvT_d, woT_d, yT_d, mask_d, ones_d):
    const = tc.alloc_tile_pool(name="const", bufs=1)
    perm = tc.alloc_tile_pool(name="perm", bufs=1)
    p01 = tc.alloc_tile_pool(name="p01", bufs=1)

    mask = const.tile([128, 128], BF16)
    ones_f32 = const.tile([128, 64], F32)
    ones128 = const.tile([128, 64], BF16)
    warm_w = const.tile([64, 64], BF16)   # garbage-free spin operand

    woT = perm.tile([128, 2, D], BF16)   # woT[p, ec, o] = wo[o, ec*128+p]
    qT = perm.tile([128, 2, S], BF16)    # qT[p, ec, s] = q[s, ec*128+p]
    kT = perm.tile([128, 2, S], BF16)
    v_sb = perm.tile([128, NKB, NH, DK + 1], BF16)  # [.., 64] = ones column

    xT = p01.tile([128, 8, S], BF16)     # xT[p, dc, s] = x[s, dc*128+p]
    wqT = p01.tile([128, 8, E], BF16)    # wqT[p, dc, e] = wq[e, dc*128+p]
    wkT = p01.tile([128, 8, E], BF16)
    wvT = p01.tile([128, 8, E], BF16)

    # spin-tile init on the DVE: the vector sequencer comes up before
    # the PE's, so the warm-up spin starts at PE-main with zero wait
    # (it MUST precede the DMA-gated ones128 copy in the DVE queue)
    nc.vector.memset(warm_w, 0.0)

    # ALL bulk inputs go on ONE queue (sync, HW DGE) in strict priority
    # order.  The host pre-arranges every tensor in its exact SBUF tile
    # layout, so each DMA is 128 contiguous 4-8KB runs (the DGE issue
    # rate of ~170 descriptors/us, not HBM bandwidth, was the gate when
    # the runs were 512B-1KB per-partition lines)
    def x_chunk(sc, half=None):
        dcs = slice(0, 8) if half is None else slice(half * 4, half * 4 + 4)
        n = 4096 if half is None else 2048
        nc.sync.dma_start(
            out=xT[:, dcs, sc * 512:(sc + 1) * 512],
            in_=bass.AP(
                tensor=xT_d.tensor,
                offset=sc * 128 * 4096 + (0 if half is None else half * 2048),
                ap=[[4096, 128], [1, n]],
            ),
        )

    # wq on the scalar queue (HW DGE, otherwise idle) so it lands in
    # parallel with xT sc0 from the sync queue; both in dc-halves so the
    # first projection chain starts on dc0-3 while dc4-7 is in flight
    nc.scalar.dma_start(out=wqT[:, 0:4, :], in_=wqT_d[:, 0:4, :])
    nc.scalar.dma_start(out=wqT[:, 4:8, :], in_=wqT_d[:, 4:8, :])
    x_chunk(0, 0)
    x_chunk(0, 1)
    nc.sync.dma_start(out=wkT[:, 0:4, :], in_=wkT_d[:, 0:4, :])
    nc.sync.dma_start(out=wkT[:, 4:8, :], in_=wkT_d[:, 4:8, :])
    nc.sync.dma_start(out=wvT, in_=wvT_d)
    x_chunk(1)
    nc.sync.dma_start(out=woT, in_=woT_d)
    x_chunk(2)
    x_chunk(3)
    nc.gpsimd.dma_start(out=mask, in_=mask_d)
    nc.gpsimd.dma_start(out=ones_f32, in_=ones_d)
    # ones row for the rowsum broadcast (row 64 used as lhsT)
    nc.vector.tensor_copy(ones128, ones_f32)

    def copy(dst, src):
        # ACT is reserved for the exp stream (it paces the attention
        # windows): all psum->sbuf copies go to the DVE
        nc.vector.tensor_copy(dst, src)

    work = tc.alloc_tile_pool(name="work", bufs=3)
    small = tc.alloc_tile_pool(name="small", bufs=2)

    # ---- phases 1-3 fused: the attention windows are exp(ACT)-paced, so
    # the q/k/v projections (pure PE work) drain INTO the windows as
    # background fuel; window qc only needs proj chunks sc <= qc ----
    with tc.tile_pool(name="ps01", bufs=1, space="PSUM") as ps01, \
         tc.tile_pool(name="psS", bufs=1, space="PSUM") as ps_S, \
         tc.tile_pool(name="psO", bufs=1, space="PSUM") as ps_o:
        ps_y = ps01

        # HAM warm-up: the PE clock-gate defaults to 4/8 (1.2 GHz) and
        # only opens to 8/8 after ~3.4us of sustained activity.  The
        # first ~4us of the kernel is DMA wait anyway, so spin small
        # matmuls on a zeroed tile: by the time wq/xT land, the PE
        # runs the real projections at full clock instead of half.
        warm_ps = ps01.tile([64, 64], F32, tag="y", bufs=2, name="warmps")
        for _ in range(80):
            nc.tensor.matmul(warm_ps, lhsT=warm_w, rhs=warm_w, start=True, stop=True)

        # ones column of v (written once; strided 3D AP)
        ones_ap = bass.AP(
            tensor=v_sb.tensor,
            offset=v_sb.offset + DK,
            ap=[v_sb.ap[0], [NH * (DK + 1), NKB], [DK + 1, NH]],
        )
        src64 = bass.AP(
            tensor=ones_f32.tensor, offset=ones_f32.offset,
            ap=[ones_f32.ap[0], [4, NKB], [1, NH]],
        )
        nc.vector.tensor_copy(ones_ap, src64)

        # touch exp once so the ~2.7us ACT table load happens during the
        # projection warm-up instead of stalling the first QK window
        warm = const.tile([1, 4], F32)
        nc.scalar.activation(
            warm, ones_f32[0:1, 0:4], mybir.ActivationFunctionType.Exp
        )

        def make_proj(w_t, outT, ec, sc):
            def u():
                ps = ps01.tile([128, 512], F32, tag="y", bufs=2, name="psp")
                for dc in range(8):
                    nc.tensor.matmul(
                        ps,
                        lhsT=w_t[:, dc, ec * 128:(ec + 1) * 128],
                        rhs=xT[:, dc, sc * 512:(sc + 1) * 512],
                        start=(dc == 0),
                        stop=(dc == 7),
                    )
                copy(outT[:, ec, sc * 512:(sc + 1) * 512], ps)
            return u

        # only the (sc0, ec0) q/k chunks are needed before window 0's
        # hp0 chain: emit those inline, everything else becomes fuel
        # (the ec1 chunks are only read by each window's hp1 chain)
        projq = []
        for sc in range(4):
            for w_t, outT in [(wqT, qT), (wkT, kT)]:
                for ec in range(2):
                    if sc == 0 and ec == 0:
                        make_proj(w_t, outT, ec, sc)()
                    else:
                        projq.append((sc, ec, make_proj(w_t, outT, ec, sc)))

        # warm-clock PE-ns cost estimates per unit kind, used to pace the
        # drain in time units (a unit-count budget lets cheap units bunch
        # and starve the ACT exp stream at window boundaries)
        PROJ_NS = 1730     # 8 matmuls N=512
        VPROJ_NS = 880     # 8 matmuls N=256
        AV_NS = 500        # matmul pair N<=512
        OUTPROJ_NS = 430   # 2 matmuls N=512
        EPI_NS = 300       # bc matmul pair

        # (cost_ns, min_slot, closure): a unit may only drain once
        # slot_i >= min_slot (keeps AV >= 3 QK slots behind its exp, and
        # defers vproj(kb) to the first window that consumes it)
        workq = []
        slot_i = [0]
        fuel_ns = [len(projq) * PROJ_NS]

        def make_vproj(sblk):
            def u():
                ps = ps01.tile([128, E], F32, tag="y", bufs=2, name="psv")
                for dc in range(8):
                    nc.tensor.matmul(
                        ps,
                        lhsT=xT[:, dc, sblk * 128:(sblk + 1) * 128],
                        rhs=wvT[:, dc, :],
                        start=(dc == 0),
                        stop=(dc == 7),
                    )
                # scatter 4 heads into [.., l, 0:64]
                sap = bass.AP(
                    tensor=ps.tensor, offset=ps.offset,
                    ap=[ps.ap[0], [DK, NH], [1, DK]],
                )
                nc.vector.tensor_copy(v_sb[:, sblk, :, 0:DK], sap)
            return u

        # attention windows (q-column ranges)
        WINS = [(0, 512), (512, 512), (1024, 512), (1536, 512)]
        _starts = []
        _acc = 0
        for (_q0, _w) in WINS:
            _starts.append(_acc)
            _acc += 2 * ((_q0 + _w) // 128)
        TOTAL_SLOTS = _acc                                           # 108

        def _vp_start(kb):
            for (s, (_q0, _w)) in zip(_starts, WINS):
                if kb * 128 < _q0 + _w:
                    return s
            return 0

        for sblk in range(NKB):
            workq.append((VPROJ_NS, _vp_start(sblk), make_vproj(sblk)))
            fuel_ns[0] += VPROJ_NS

        # drain pacing: spread the queued PE work evenly (in estimated
        # ns) over the remaining QK slots, so fuel neither bunches into
        # ACT-starving bursts nor runs dry before the last window; scan
        # past not-yet-eligible units (safe: relative order of dependent
        # units is preserved by min_slot construction)
        def drain_some():
            slots_left = max(1, TOTAL_SLOTS - slot_i[0])
            target = min(3400, fuel_ns[0] // slots_left)
            spent = 0
            j = 0
            while j < len(workq) and spent < target:
                if workq[j][1] <= slot_i[0]:
                    c, _, u = workq.pop(j)
                    u()
                    fuel_ns[0] -= c
                    spent += c
                else:
                    j += 1
            # proj chunk sc's xT arrives ~2-3us per chunk after kernel
            # start: don't pop it as fuel before its data can be there
            PROJ_GATE = {0: 0, 1: 0, 2: 2, 3: 4}
            if spent < target and projq and slot_i[0] >= PROJ_GATE[projq[0][0]]:
                _, _, u = projq.pop(0)
                u()
                fuel_ns[0] -= PROJ_NS

        def make_av(po_box, pts, kb, hp, kmax, width, lw=False):
            last = kb == kmax - 1

            def av():
                if po_box[0] is None:
                    po_box[0] = (
                        ps_o.tile([DK + 1, QC], F32, tag="o", bufs=2, name="poA"),
                        ps_o.tile([DK + 1, QC], F32, tag="o", bufs=2, name="poB"),
                    )
                poA, poB = po_box[0]
                pT, cs = pts[kb]
                for hi, po in ((0, poA), (1, poB)):
                    nc.tensor.matmul(
                        po[:, cs:width],
                        lhsT=v_sb[:, kb, 2 * hp + hi, :],
                        rhs=pT[:, hi, cs:width],
                        start=(kb == 0),
                        stop=last,
                    )
                if last:
                    # stage the epilogue inputs: both heads' o^T copied
                    # out to SBUF immediately so the poA/poB PSUM banks
                    # free HERE (the next head-pair's first AVs wait on
                    # this ring — leaving poA to be read by the epilogue
                    # mul kept the bank busy ~2.5us longer).  rsA goes on
                    # ACT (idle between exps, tiny) so bcA isn't queued
                    # behind the oB copy on the DVE.
                    rsA = small.tile([1, QC], BF16, tag="rsA", bufs=2)
                    oA_sb = small.tile([DK, QC], BF16, tag="oAsb", bufs=2)
                    oB_sb = small.tile([DK + 1, QC], BF16, tag="osb", bufs=4)
                    nc.scalar.copy(rsA[:, 0:width], poA[DK:DK + 1, 0:width])
                    nc.vector.tensor_copy(oA_sb[:, 0:width], poA[0:DK, 0:width])
                    nc.vector.tensor_copy(oB_sb[:, 0:width], poB[:, 0:width])
                    oB_hi = small.tile([128, QC], BF16, tag="oBhi", bufs=2)
                    nc.gpsimd.dma_start(
                        out=oB_hi[64:128, 0:width], in_=oB_sb[0:DK, 0:width]
                    )
                    po_box[1] = (rsA, oA_sb, oB_sb, oB_hi)
            return av

        def make_epilogue(po_box, oT, hp, width):
            def epi():
                rsA, oA_sb, oB_sb, oB_hi = po_box[1]
                # both rowsums broadcast into one psum bank (head A to
                # partitions 0-63, head B to 64-127) so a single
                # 128-partition reciprocal feeds both divides; head B's
                # o^T was already shifted to partitions 64-127 right
                # after its last AV, off this critical path
                ps_bc = ps_y.tile([128, QC], F32, tag="y", bufs=2, name="psbc")
                nc.tensor.matmul(
                    ps_bc[0:64, 0:width],
                    lhsT=ones128[0:1, :],
                    rhs=rsA[:, 0:width],
                    start=True,
                    stop=True,
                )
                nc.tensor.matmul(
                    ps_bc[64:128, 0:width],
                    lhsT=ones128[64:65, :],
                    rhs=oB_sb[DK:DK + 1, 0:width],
                    start=True,
                    stop=True,
                )
                # split the reciprocal and run the two normalize
                # multiplies on different engines (DVE + GpSimd) so the
                # epilogue critical path is one recip + one mul, not a
                # serial recip->mulA->mulB chain on the DVE
                rec = small.tile([128, QC], F32, tag="rec", bufs=2)
                nc.vector.reciprocal_approx_fast(rec[:, 0:width], ps_bc[:, 0:width])
                nc.vector.tensor_mul(
                    oT[0:64, hp, 0:width], oA_sb[:, 0:width], rec[0:64, 0:width]
                )
                nc.vector.tensor_mul(
                    oT[64:128, hp, 0:width],
                    oB_hi[64:128, 0:width],
                    rec[64:128, 0:width],
                )
            return epi

        def make_out_proj(wi, q0, width, is_last, oT):
            # one window-wide staging tile and TWO output DMAs (4KB
            # per-partition runs) instead of 8 per-dc DMAs with 1KB
            # lines: the output path was DGE-issue-bound (~170
            # descriptors/us), which dominated the kernel tail
            y_all = work.tile([128, 8, QC], BF16, tag="ysb", bufs=2)
            box = {}
            units = []
            for dc in range(8):
                def u(dc=dc, oT=oT):
                    if is_last:
                        # final window: QK is done, so the score banks are
                        # free — alternate units between the S ring and
                        # the y ring so FOUR accumulations are in flight
                        # and the tail isn't copy-throttled
                        if dc % 2 == 0:
                            psyt = ps_S.tile([128, 2, 512], F32, tag="S",
                                             bufs=2, name="psyt")
                            psy = psyt[:, 0, 0:width]
                        else:
                            psyf = ps_y.tile([128, QC], F32, tag="y", bufs=2,
                                             name="psy")
                            psy = psyf[:, 0:width]
                    else:
                        psyf = ps_y.tile([128, QC], F32, tag="y", bufs=2,
                                         name="psy")
                        psy = psyf[:, 0:width]
                    for ec in range(2):
                        nc.tensor.matmul(
                            psy,
                            lhsT=woT[:, ec, dc * 128:(dc + 1) * 128],
                            rhs=oT[:, ec, 0:width],
                            start=(ec == 0),
                            stop=(ec == 1),
                        )
                    # parity split: consecutive units' copies alternate
                    # engines so the tail drains two copies at a time
                    if dc % 2 == 0:
                        nc.vector.tensor_copy(y_all[:, dc, 0:width], psy)
                    else:
                        nc.scalar.copy(y_all[:, dc, 0:width], psy)
                    # finer DMA granularity in the last window so the
                    # final (chain-ending) transfer is 256KB, not 512KB
                    step = 2 if is_last else 4
                    if dc % step == step - 1:
                        h0 = dc - step + 1
                        nc.sync.dma_start(
                            out=bass.AP(
                                tensor=yT_d.tensor,
                                offset=wi * 128 * 4096 + h0 * 512,
                                ap=[[4096, 128], [512, step], [1, 512]],
                            ),
                            in_=y_all[:, h0:dc + 1, 0:width],
                        )
                units.append(u)
            return units

        for wi, (q0, width) in enumerate(WINS):
            sc_need = (q0 + width - 1) // 512
            oT = work.tile([128, 2, QC], BF16, tag="oT", bufs=2)
            kmax = (q0 + width) // 128
            is_last = wi == len(WINS) - 1
            for hp in range(2):
                # this hp chain reads q/k chunks (sc <= sc_need, ec <= hp):
                # force any not yet drained (usually already gone as fuel)
                ii = 0
                while ii < len(projq):
                    sc_, ec_, fn_ = projq[ii]
                    if sc_ <= sc_need and ec_ <= hp:
                        projq.pop(ii)
                        fn_()
                        fuel_ns[0] -= PROJ_NS
                    else:
                        ii += 1
                pts = {}
                po_box = [None, None]
                for kb in range(kmax):
                    # drain BEFORE the QK pair: the drained AV/fuel MMs are
                    # long (213-432ns), so both their LDWEIGHTS and the QK
                    # pair's hide under preceding streams.  (With QK first,
                    # the AV's LDW lands right after the 2nd row-tiled QK
                    # matmul — which finishes ~3ns after the 1st — and is
                    # fully exposed, ~118ns per slot.)
                    drain_some()
                    # S^T = k q^T, 2-head row-tiled pair, causally narrowed
                    cs = max(0, kb * 128 - q0)
                    psS = ps_S.tile([128, 2, 512], F32, tag="S", bufs=2)
                    for hi in range(2):
                        nc.tensor.matmul(
                            psS[:, hi, cs:width],
                            lhsT=kT[hi * 64:(hi + 1) * 64, hp,
                                    kb * 128:(kb + 1) * 128],
                            rhs=qT[hi * 64:(hi + 1) * 64, hp,
                                   q0 + cs:q0 + width],
                            start=True,
                            stop=True,
                        )
                    pT = work.tile([128, 2, 512], BF16, tag="pT", bufs=32)
                    pts[kb] = (pT, cs)
                    nc.scalar.activation(
                        pT[:, :, cs:width],
                        psS[:, :, cs:width],
                        mybir.ActivationFunctionType.Exp,
                        scale=SCALE,
                    )
                    if kb * 128 >= q0:  # diagonal band: zero upper triangle
                        # 0/1 multiply AFTER exp, on the deeply-buffered pT;
                        # one op for both heads via a stride-0 middle dim.
                        # On GpSimd: the DVE is loaded with psum->sbuf
                        # copies and ACT must stay exp-only
                        mask2 = bass.AP(
                            tensor=mask.tensor, offset=mask.offset,
                            ap=[mask.ap[0], [0, 2], mask.ap[1]],
                        )
                        nc.gpsimd.tensor_mul(
                            pT[:, :, cs:cs + 128],
                            pT[:, :, cs:cs + 128],
                            mask2,
                        )
                    # AV for this kb becomes available 3 QK slots later
                    workq.append(
                        (AV_NS, slot_i[0] + 3,
                         make_av(po_box, pts, kb, hp, kmax, width, lw=is_last))
                    )
                    fuel_ns[0] += AV_NS
                    slot_i[0] += 1
                # epi min matches its last AV's eligibility; outproj one
                # later (list order breaks the tie in favor of epi)
                workq.append((EPI_NS, slot_i[0] + 2, make_epilogue(po_box, oT, hp, width)))
                fuel_ns[0] += EPI_NS
                if hp == 1:
                    units = make_out_proj(wi, q0, width, is_last, oT)
                    # reserve half of the second-to-last window's outproj
                    # as tail fuel: at the very end the PE would otherwise
                    # idle ~2.7us behind the exp->AV->epilogue chain of the
                    # final window (these units are independent of it)
                    reserve = 6 if wi == len(WINS) - 2 else 0
                    for ui, u in enumerate(units):
                        ms = 10 ** 9 if ui >= len(units) - reserve else slot_i[0] + 3
                        workq.append((OUTPROJ_NS, ms, u))
                    fuel_ns[0] += OUTPROJ_NS * len(units)
        # final flush.  The tail dependency chain is last-exp -> last-AVs
        # -> epilogue copies -> bc/recip/muls -> final outproj; the
        # reserved units (min_slot sentinel) are independent of all of
        # it, so emit them BETWEEN the AVs and the epilogue where the PE
        # would otherwise idle ~3us behind the DVE/ACT epilogue chain.
        reserved = [e for e in workq if e[1] >= 10 ** 8]
        rest = [e for e in workq if e[1] < 10 ** 8]
        last_epi = max(
            (i for i, e in enumerate(rest) if e[0] == EPI_NS), default=0
        )
        # 2 reserved units cover the epilogue-staging copies, the rest
        # cover the bc->recip->mul DVE chain (~2.5us) that the final
        # outproj waits on
        for _, _, u in rest[:last_epi + 1]:
            u()
        for _, _, u in reserved:
            u()
        for _, _, u in rest[last_epi + 1:]:
            u()

    for p in [small, work, p01, perm, const]:
        p.release()


_CACHE = {}


def _build():
    if "nc" in _CACHE:
        return _CACHE["nc"]
    nc = bacc.Bacc("TRN2", target_bir_lowering=False, debug=False, num_devices=8)
    # all inputs pre-arranged host-side into their SBUF tile layouts so
    # every DMA lands as 128 contiguous 4-8KB per-partition runs
    xT_d = nc.dram_tensor("xT", [4, 128, 4096], BF16, kind="ExternalInput").ap()
    wqT_d = nc.dram_tensor("wqT", [128, 8, E], BF16, kind="ExternalInput").ap()
    wkT_d = nc.dram_tensor("wkT", [128, 8, E], BF16, kind="ExternalInput").ap()
    wvT_d = nc.dram_tensor("wvT", [128, 8, E], BF16, kind="ExternalInput").ap()
    woT_d = nc.dram_tensor("woT", [128, 2, D], BF16, kind="ExternalInput").ap()
    yT_d = nc.dram_tensor("yT", [4, 128, 4096], BF16, kind="ExternalOutput").ap()
    mask_d = nc.dram_tensor("maskc", [128, 128], BF16, kind="ExternalInput").ap()
    ones_d = nc.dram_tensor("onesc", [128, 64], F32, kind="ExternalInput").ap()
    with tile.TileContext(nc) as tc:
        _emit(tc, nc, xT_d, wqT_d, wkT_d, wvT_d, woT_d, yT_d, mask_d, ones_d)
    nc.compile()
    _CACHE["nc"] = nc
    return nc


_r = np.arange(128)
_MASK = np.where(_r[:, None] <= _r[None, :], 1.0, 0.0).astype(ml_dtypes.bfloat16)
_ONES = np.ones((128, 64), dtype=np.float32)

LAST_RESULT = None


def kernel(x, wq, wk, wv, wo):
    global LAST_RESULT
    nc = _build()
    bf = ml_dtypes.bfloat16
    x = np.asarray(x, dtype=np.float32)
    wq = np.asarray(wq, dtype=np.float32)
    wk = np.asarray(wk, dtype=np.float32)
    wv = np.asarray(wv, dtype=np.float32)
    wo = np.asarray(wo, dtype=np.float32)

    def wlay(w2d):  # [256 out, 1024 in] -> [p, dc, e] = w2d[e, dc*128+p]
        return np.ascontiguousarray(
            w2d.T.reshape(8, 128, 256).transpose(1, 0, 2).astype(bf))

    in_maps = []
    for c in range(8):
        b, g = c // 4, c % 4
        rows = slice(g * E, (g + 1) * E)
        # xT dram [sc, p, dc*512+c] = x[sc*512+c, dc*128+p]
        xr = x[b].reshape(4, 512, 8, 128).transpose(0, 3, 2, 1).reshape(4, 128, 4096)
        # woT dram [p, ec, o] = wo[o, ec*128+p (within rows)]
        wor = wo[:, rows].T.reshape(2, 128, 1024).transpose(1, 0, 2)
        in_maps.append({
            "xT": np.ascontiguousarray(xr.astype(bf)),
            "wqT": wlay(wq[rows]),
            "wkT": wlay(wk[rows]),
            "wvT": wlay(wv[rows]),
            "woT": np.ascontiguousarray(wor.astype(bf)),
            "maskc": _MASK,
            "onesc": _ONES,
        })

    res = bass_utils.run_bass_kernel_spmd(nc, in_maps, core_ids=list(range(8)))
    LAST_RESULT = res

    y = np.empty((B, S, D), dtype=np.float32)
    for b in range(B):
        acc = res.results[4 * b]["yT"].astype(np.float32)
        for g in range(1, 4):
            acc += res.results[4 * b + g]["yT"].astype(np.float32)
        # yT dram [wi, p, dc*512+c] = y[wi*512+c, dc*128+p]
        y[b] = acc.reshape(4, 128, 8, 512).transpose(0, 3, 2, 1).reshape(S, D)
    return y



# revision 73
# speedup vs baseline: 1.0226x; 1.0073x over previous
"""Causal multi-head attention on 8 Trainium2 NeuronCores.

Sharding: data-parallel over batch (B=2) x tensor-parallel over heads
(16 heads -> 4 groups of 4). Core c handles batch c//4, head group c%4.
Each core computes q/k/v projections for its 4 heads, causal flash
attention, and a partial output projection (row slice of Wo); the host
sums the 4 partials per batch element.

All transposes happen on the HOST, and every input is pre-arranged in
its exact SBUF tile layout so each DMA lands as 128 contiguous 4-8KB
per-partition runs (the DGE issues ~170 descriptors/us, so descriptor
COUNT — not HBM bandwidth — gates the load when runs are 512B-1KB).
All bulk inputs stream on one HW-DGE queue in priority order (wq, x
sc0, wk, wv, x sc1, wo, x sc2-3); the output y^T is staged per window
and leaves in 2-4 wide DMAs per window for the same reason.

Matmuls run in bf16 (fp32 PSUM accumulation). The softmax row-sum is
fused into the o^T = [v|1s]^T P^T matmul via an appended ones column;
normalization (broadcast rowsum via K=1 matmul, fast-approx
reciprocal, divide) stays in fp32.

Engine assignment: ACT runs the exp stream and nothing else mid-window
(it co-paces the attention slots with the PE); DVE takes all
psum->sbuf copies + reciprocal; GpSimd takes the causal-mask
multiplies and v-scatters.  A ~100-matmul warm-up spin on a zeroed
tile runs during the initial DMA wait so the PE HAM clock-gate is at
8/8 (2.4 GHz) when real work starts.

Scheduling: per (q-chunk, head-pair) only the row-tiled S^T = k q^T
pair and the masked exp are emitted in the main loop; everything else
(projection chunks, AV matmuls, epilogues, output projections) drains
from a work queue between QK slots, paced in estimated PE-ns so fuel
neither bunches into ACT-starving bursts nor runs dry before the last
window (either one lets the HAM re-throttle to half clock).  Six
output-projection units of the second-to-last window are reserved and
replayed at the very end, where the PE would otherwise idle ~3us
behind the final epilogue's DVE chain.
"""

import numpy as np
import ml_dtypes

import concourse.bacc as bacc
import concourse.bass as bass
import concourse.tile as tile
from concourse import bass_utils, mybir

B, S, D, H = 2, 2048, 1024, 16
DK = 64
NH = 4                 # heads per core
E = NH * DK            # 256: per-core head-dim slice
SCALE = 1.0 / 8.0      # 1/sqrt(DK)

F32 = mybir.dt.float32
F32R = mybir.dt.float32r
BF16 = mybir.dt.bfloat16

QC = 512               # q-chunk (columns per attention tile)
NQC = S // QC          # 4
NKB = S // 128         # 16 k-blocks


def _emit(tc, nc, xT_d, wqT_d, wkT_d, wvT_d, woT_d, yT_d, mask_d, ones_d):
    const = tc.alloc_tile_pool(name="const", bufs=1)
    perm = tc.alloc_tile_pool(name="perm", bufs=1)
    p01 = tc.alloc_tile_pool(name="p01", bufs=1)

    mask = const.tile([128, 128], BF16)
    ones_f32 = const.tile([128, 64], F32)
    ones128 = const.tile([128, 64], BF16)
    warm_w = const.tile([64, 64], BF16)   # garbage-free spin operand

    woT = perm.tile([128, 2, D], BF16)   # woT[p, ec, o] = wo[o, ec*128+p]
    qT = perm.tile([128, 2, S], BF16)    # qT[p, ec, s] = q[s, ec*128+p]
    kT = perm.tile([128, 2, S], BF16)
    v_sb = perm.tile([128, NKB, NH, DK + 1], BF16)  # [.., 64] = ones column

    xT = p01.tile([128, 8, S], BF16)     # xT[p, dc, s] = x[s, dc*128+p]
    wqT = p01.tile([128, 8, E], BF16)    # wqT[p, dc, e] = wq[e, dc*128+p]
    wkT = p01.tile([128, 8, E], BF16)
    wvT = p01.tile([128, 8, E], BF16)

    # spin-tile init on the DVE: the vector sequencer comes up before
    # the PE's, so the warm-up spin starts at PE-main with zero wait
    # (it MUST precede the DMA-gated ones128 copy in the DVE queue)
    nc.vector.memset(warm_w, 0.0)

    # ALL bulk inputs go on ONE queue (sync, HW DGE) in strict priority
    # order.  The host pre-arranges every tensor in its exact SBUF tile
    # layout, so each DMA is 128 contiguous 4-8KB runs (the DGE issue
    # rate of ~170 descriptors/us, not HBM bandwidth, was the gate when
    # the runs were 512B-1KB per-partition lines)
    def x_chunk(sc, half=None):
        dcs = slice(0, 8) if half is None else slice(half * 4, half * 4 + 4)
        n = 4096 if half is None else 2048
        nc.sync.dma_start(
            out=xT[:, dcs, sc * 512:(sc + 1) * 512],
            in_=bass.AP(
                tensor=xT_d.tensor,
                offset=sc * 128 * 4096 + (0 if half is None else half * 2048),
                ap=[[4096, 128], [1, n]],
            ),
        )

    # wq on the scalar queue (HW DGE, otherwise idle) so it lands in
    # parallel with xT sc0 from the sync queue; both in dc-halves so the
    # first projection chain starts on dc0-3 while dc4-7 is in flight
    nc.scalar.dma_start(out=wqT[:, 0:4, :], in_=wqT_d[:, 0:4, :])
    nc.scalar.dma_start(out=wqT[:, 4:8, :], in_=wqT_d[:, 4:8, :])
    x_chunk(0, 0)
    x_chunk(0, 1)
    nc.sync.dma_start(out=wkT[:, 0:4, :], in_=wkT_d[:, 0:4, :])
    nc.sync.dma_start(out=wkT[:, 4:8, :], in_=wkT_d[:, 4:8, :])
    nc.sync.dma_start(out=wvT, in_=wvT_d)
    x_chunk(1)
    nc.sync.dma_start(out=woT, in_=woT_d)
    x_chunk(2)
    x_chunk(3)
    nc.gpsimd.dma_start(out=mask, in_=mask_d)
    nc.gpsimd.dma_start(out=ones_f32, in_=ones_d)
    # ones row for the rowsum broadcast (row 64 used as lhsT)
    nc.vector.tensor_copy(ones128, ones_f32)

    def copy(dst, src):
        # ACT is reserved for the exp stream (it paces the attention
        # windows): all psum->sbuf copies go to the DVE
        nc.vector.tensor_copy(dst, src)

    work = tc.alloc_tile_pool(name="work", bufs=3)
    small = tc.alloc_tile_pool(name="small", bufs=2)

    # ---- phases 1-3 fused: the attention windows are exp(ACT)-paced, so
    # the q/k/v projections (pure PE work) drain INTO the windows as
    # background fuel; window qc only needs proj chunks sc <= qc ----
    with tc.tile_pool(name="ps01", bufs=1, space="PSUM") as ps01, \
         tc.tile_pool(name="psS", bufs=1, space="PSUM") as ps_S, \
         tc.tile_pool(name="psO", bufs=1, space="PSUM") as ps_o:
        ps_y = ps01

        # HAM warm-up: the PE clock-gate defaults to 4/8 (1.2 GHz) and
        # only opens to 8/8 after ~3.4us of sustained activity.  The
        # first ~4us of the kernel is DMA wait anyway, so spin small
        # matmuls on a zeroed tile: by the time wq/xT land, the PE
        # runs the real projections at full clock instead of half.
        warm_ps = ps01.tile([64, 64], F32, tag="y", bufs=2, name="warmps")
        for _ in range(80):
            nc.tensor.matmul(warm_ps, lhsT=warm_w, rhs=warm_w, start=True, stop=True)

        # ones column of v (written once; strided 3D AP)
        ones_ap = bass.AP(
            tensor=v_sb.tensor,
            offset=v_sb.offset + DK,
            ap=[v_sb.ap[0], [NH * (DK + 1), NKB], [DK + 1, NH]],
        )
        src64 = bass.AP(
            tensor=ones_f32.tensor, offset=ones_f32.offset,
            ap=[ones_f32.ap[0], [4, NKB], [1, NH]],
        )
        nc.vector.tensor_copy(ones_ap, src64)

        # touch exp once so the ~2.7us ACT table load happens during the
        # projection warm-up instead of stalling the first QK window
        warm = const.tile([1, 4], F32)
        nc.scalar.activation(
            warm, ones_f32[0:1, 0:4], mybir.ActivationFunctionType.Exp
        )

        def make_proj(w_t, outT, ec, sc):
            def u():
                ps = ps01.tile([128, 512], F32, tag="y", bufs=2, name="psp")
                for dc in range(8):
                    nc.tensor.matmul(
                        ps,
                        lhsT=w_t[:, dc, ec * 128:(ec + 1) * 128],
                        rhs=xT[:, dc, sc * 512:(sc + 1) * 512],
                        start=(dc == 0),
                        stop=(dc == 7),
                    )
                copy(outT[:, ec, sc * 512:(sc + 1) * 512], ps)
            return u

        # only the (sc0, ec0) q/k chunks are needed before window 0's
        # hp0 chain: emit those inline, everything else becomes fuel
        # (the ec1 chunks are only read by each window's hp1 chain)
        projq = []
        for sc in range(4):
            for w_t, outT in [(wqT, qT), (wkT, kT)]:
                for ec in range(2):
                    if sc == 0 and ec == 0:
                        make_proj(w_t, outT, ec, sc)()
                    else:
                        projq.append((sc, ec, make_proj(w_t, outT, ec, sc)))

        # warm-clock PE-ns cost estimates per unit kind, used to pace the
        # drain in time units (a unit-count budget lets cheap units bunch
        # and starve the ACT exp stream at window boundaries)
        PROJ_NS = 1730     # 8 matmuls N=512
        VPROJ_NS = 880     # 8 matmuls N=256
        AV_NS = 500        # matmul pair N<=512
        OUTPROJ_NS = 430   # 2 matmuls N=512
        EPI_NS = 300       # bc matmul pair

        # (cost_ns, min_slot, closure): a unit may only drain once
        # slot_i >= min_slot (keeps AV >= 3 QK slots behind its exp, and
        # defers vproj(kb) to the first window that consumes it)
        workq = []
        slot_i = [0]
        fuel_ns = [len(projq) * PROJ_NS]

        def make_vproj(sblk):
            def u():
                ps = ps01.tile([128, E], F32, tag="y", bufs=2, name="psv")
                for dc in range(8):
                    nc.tensor.matmul(
                        ps,
                        lhsT=xT[:, dc, sblk * 128:(sblk + 1) * 128],
                        rhs=wvT[:, dc, :],
                        start=(dc == 0),
                        stop=(dc == 7),
                    )
                # scatter 4 heads into [.., l, 0:64] as four contiguous
                # 2D copies: a single strided copy gets auto-routed to a
                # ~660ns GpSimd DIRECT2D that queues behind the causal
                # mask multiplies — and the AV matmuls wait on v_sb
                for l in range(NH):
                    nc.vector.tensor_copy(
                        v_sb[:, sblk, l, 0:DK], ps[:, l * DK:(l + 1) * DK]
                    )
            return u

        # attention windows (q-column ranges)
        WINS = [(0, 512), (512, 512), (1024, 512), (1536, 512)]
        _starts = []
        _acc = 0
        for (_q0, _w) in WINS:
            _starts.append(_acc)
            _acc += 2 * ((_q0 + _w) // 128)
        TOTAL_SLOTS = _acc                                           # 108

        def _vp_start(kb):
            for (s, (_q0, _w)) in zip(_starts, WINS):
                if kb * 128 < _q0 + _w:
                    return s
            return 0

        for sblk in range(NKB):
            workq.append((VPROJ_NS, _vp_start(sblk), make_vproj(sblk)))
            fuel_ns[0] += VPROJ_NS

        # drain pacing: spread the queued PE work evenly (in estimated
        # ns) over the remaining QK slots, so fuel neither bunches into
        # ACT-starving bursts nor runs dry before the last window; scan
        # past not-yet-eligible units (safe: relative order of dependent
        # units is preserved by min_slot construction)
        def drain_some():
            slots_left = max(1, TOTAL_SLOTS - slot_i[0])
            target = min(3400, fuel_ns[0] // slots_left)
            spent = 0
            j = 0
            while j < len(workq) and spent < target:
                if workq[j][1] <= slot_i[0]:
                    c, _, u = workq.pop(j)
                    u()
                    fuel_ns[0] -= c
                    spent += c
                else:
                    j += 1
            # proj chunk sc's xT arrives ~2-3us per chunk after kernel
            # start: don't pop it as fuel before its data can be there
            PROJ_GATE = {0: 0, 1: 0, 2: 2, 3: 4}
            if spent < target and projq and slot_i[0] >= PROJ_GATE[projq[0][0]]:
                _, _, u = projq.pop(0)
                u()
                fuel_ns[0] -= PROJ_NS

        def make_av(po_box, pts, kb, hp, kmax, width, lw=False):
            last = kb == kmax - 1

            def av():
                if po_box[0] is None:
                    po_box[0] = (
                        ps_o.tile([DK + 1, QC], F32, tag="o", bufs=2, name="poA"),
                        ps_o.tile([DK + 1, QC], F32, tag="o", bufs=2, name="poB"),
                    )
                poA, poB = po_box[0]
                pT, cs = pts[kb]
                for hi, po in ((0, poA), (1, poB)):
                    nc.tensor.matmul(
                        po[:, cs:width],
                        lhsT=v_sb[:, kb, 2 * hp + hi, :],
                        rhs=pT[:, hi, cs:width],
                        start=(kb == 0),
                        stop=last,
                    )
                if last:
                    # stage the epilogue inputs: both heads' o^T copied
                    # out to SBUF immediately so the poA/poB PSUM banks
                    # free HERE (the next head-pair's first AVs wait on
                    # this ring — leaving poA to be read by the epilogue
                    # mul kept the bank busy ~2.5us longer).  rsA goes on
                    # ACT (idle between exps, tiny) so bcA isn't queued
                    # behind the oB copy on the DVE.
                    rsA = small.tile([1, QC], BF16, tag="rsA", bufs=2)
                    oA_sb = small.tile([DK, QC], BF16, tag="oAsb", bufs=2)
                    oB_sb = small.tile([DK + 1, QC], BF16, tag="osb", bufs=4)
                    nc.scalar.copy(rsA[:, 0:width], poA[DK:DK + 1, 0:width])
                    nc.vector.tensor_copy(oA_sb[:, 0:width], poA[0:DK, 0:width])
                    nc.vector.tensor_copy(oB_sb[:, 0:width], poB[:, 0:width])
                    oB_hi = small.tile([128, QC], BF16, tag="oBhi", bufs=2)
                    nc.gpsimd.dma_start(
                        out=oB_hi[64:128, 0:width], in_=oB_sb[0:DK, 0:width]
                    )
                    po_box[1] = (rsA, oA_sb, oB_sb, oB_hi)
            return av

        def make_epilogue(po_box, oT, hp, width):
            def epi():
                rsA, oA_sb, oB_sb, oB_hi = po_box[1]
                # both rowsums broadcast into one psum bank (head A to
                # partitions 0-63, head B to 64-127) so a single
                # 128-partition reciprocal feeds both divides; head B's
                # o^T was already shifted to partitions 64-127 right
                # after its last AV, off this critical path
                ps_bc = ps_y.tile([128, QC], F32, tag="y", bufs=2, name="psbc")
                nc.tensor.matmul(
                    ps_bc[0:64, 0:width],
                    lhsT=ones128[0:1, :],
                    rhs=rsA[:, 0:width],
                    start=True,
                    stop=True,
                )
                nc.tensor.matmul(
                    ps_bc[64:128, 0:width],
                    lhsT=ones128[64:65, :],
                    rhs=oB_sb[DK:DK + 1, 0:width],
                    start=True,
                    stop=True,
                )
                # split the reciprocal and run the two normalize
                # multiplies on different engines (DVE + GpSimd) so the
                # epilogue critical path is one recip + one mul, not a
                # serial recip->mulA->mulB chain on the DVE
                rec = small.tile([128, QC], F32, tag="rec", bufs=2)
                nc.vector.reciprocal_approx_fast(rec[:, 0:width], ps_bc[:, 0:width])
                nc.vector.tensor_mul(
                    oT[0:64, hp, 0:width], oA_sb[:, 0:width], rec[0:64, 0:width]
                )
                nc.vector.tensor_mul(
                    oT[64:128, hp, 0:width],
                    oB_hi[64:128, 0:width],
                    rec[64:128, 0:width],
                )
            return epi

        def make_out_proj(wi, q0, width, is_last, oT):
            # one window-wide staging tile and TWO output DMAs (4KB
            # per-partition runs) instead of 8 per-dc DMAs with 1KB
            # lines: the output path was DGE-issue-bound (~170
            # descriptors/us), which dominated the kernel tail
            y_all = work.tile([128, 8, QC], BF16, tag="ysb", bufs=2)
            box = {}
            units = []
            for dc in range(8):
                def u(dc=dc, oT=oT):
                    if is_last:
                        # final window: QK is done, so the score banks are
                        # free — alternate units between the S ring and
                        # the y ring so FOUR accumulations are in flight
                        # and the tail isn't copy-throttled
                        if dc % 2 == 0:
                            psyt = ps_S.tile([128, 2, 512], F32, tag="S",
                                             bufs=2, name="psyt")
                            psy = psyt[:, 0, 0:width]
                        else:
                            psyf = ps_y.tile([128, QC], F32, tag="y", bufs=2,
                                             name="psy")
                            psy = psyf[:, 0:width]
                    else:
                        psyf = ps_y.tile([128, QC], F32, tag="y", bufs=2,
                                         name="psy")
                        psy = psyf[:, 0:width]
                    for ec in range(2):
                        nc.tensor.matmul(
                            psy,
                            lhsT=woT[:, ec, dc * 128:(dc + 1) * 128],
                            rhs=oT[:, ec, 0:width],
                            start=(ec == 0),
                            stop=(ec == 1),
                        )
                    # parity split: consecutive units' copies alternate
                    # engines so the tail drains two copies at a time
                    if dc % 2 == 0:
                        nc.vector.tensor_copy(y_all[:, dc, 0:width], psy)
                    else:
                        nc.scalar.copy(y_all[:, dc, 0:width], psy)
                    # finer DMA granularity in the last window so the
                    # final (chain-ending) transfer is 256KB, not 512KB
                    step = 2 if is_last else 4
                    if dc % step == step - 1:
                        h0 = dc - step + 1
                        nc.sync.dma_start(
                            out=bass.AP(
                                tensor=yT_d.tensor,
                                offset=wi * 128 * 4096 + h0 * 512,
                                ap=[[4096, 128], [512, step], [1, 512]],
                            ),
                            in_=y_all[:, h0:dc + 1, 0:width],
                        )
                units.append(u)
            return units

        for wi, (q0, width) in enumerate(WINS):
            sc_need = (q0 + width - 1) // 512
            oT = work.tile([128, 2, QC], BF16, tag="oT", bufs=2)
            kmax = (q0 + width) // 128
            is_last = wi == len(WINS) - 1
            for hp in range(2):
                # this hp chain reads q/k chunks (sc <= sc_need, ec <= hp):
                # force any not yet drained (usually already gone as fuel)
                ii = 0
                while ii < len(projq):
                    sc_, ec_, fn_ = projq[ii]
                    if sc_ <= sc_need and ec_ <= hp:
                        projq.pop(ii)
                        fn_()
                        fuel_ns[0] -= PROJ_NS
                    else:
                        ii += 1
                pts = {}
                po_box = [None, None]
                for kb in range(kmax):
                    # drain BEFORE the QK pair: the drained AV/fuel MMs are
                    # long (213-432ns), so both their LDWEIGHTS and the QK
                    # pair's hide under preceding streams.  (With QK first,
                    # the AV's LDW lands right after the 2nd row-tiled QK
                    # matmul — which finishes ~3ns after the 1st — and is
                    # fully exposed, ~118ns per slot.)
                    drain_some()
                    # S^T = k q^T, 2-head row-tiled pair, causally narrowed
                    cs = max(0, kb * 128 - q0)
                    psS = ps_S.tile([128, 2, 512], F32, tag="S", bufs=2)
                    for hi in range(2):
                        nc.tensor.matmul(
                            psS[:, hi, cs:width],
                            lhsT=kT[hi * 64:(hi + 1) * 64, hp,
                                    kb * 128:(kb + 1) * 128],
                            rhs=qT[hi * 64:(hi + 1) * 64, hp,
                                   q0 + cs:q0 + width],
                            start=True,
                            stop=True,
                        )
                    pT = work.tile([128, 2, 512], BF16, tag="pT", bufs=32)
                    pts[kb] = (pT, cs)
                    nc.scalar.activation(
                        pT[:, :, cs:width],
                        psS[:, :, cs:width],
                        mybir.ActivationFunctionType.Exp,
                        scale=SCALE,
                    )
                    if kb * 128 >= q0:  # diagonal band: zero upper triangle
                        # 0/1 multiply AFTER exp, on the deeply-buffered pT;
                        # one op for both heads via a stride-0 middle dim.
                        # On GpSimd: the DVE is loaded with psum->sbuf
                        # copies and ACT must stay exp-only
                        mask2 = bass.AP(
                            tensor=mask.tensor, offset=mask.offset,
                            ap=[mask.ap[0], [0, 2], mask.ap[1]],
                        )
                        nc.gpsimd.tensor_mul(
                            pT[:, :, cs:cs + 128],
                            pT[:, :, cs:cs + 128],
                            mask2,
                        )
                    # AV for this kb becomes available 3 QK slots later
                    workq.append(
                        (AV_NS, slot_i[0] + 3,
                         make_av(po_box, pts, kb, hp, kmax, width, lw=is_last))
                    )
                    fuel_ns[0] += AV_NS
                    slot_i[0] += 1
                # epi min matches its last AV's eligibility; outproj one
                # later (list order breaks the tie in favor of epi)
                workq.append((EPI_NS, slot_i[0] + 2, make_epilogue(po_box, oT, hp, width)))
                fuel_ns[0] += EPI_NS
                if hp == 1:
                    units = make_out_proj(wi, q0, width, is_last, oT)
                    # reserve half of the second-to-last window's outproj
                    # as tail fuel: at the very end the PE would otherwise
                    # idle ~2.7us behind the exp->AV->epilogue chain of the
                    # final window (these units are independent of it)
                    reserve = 6 if wi == len(WINS) - 2 else 0
                    for ui, u in enumerate(units):
                        ms = 10 ** 9 if ui >= len(units) - reserve else slot_i[0] + 3
                        workq.append((OUTPROJ_NS, ms, u))
                    fuel_ns[0] += OUTPROJ_NS * len(units)
        # final flush.  The tail dependency chain is last-exp -> last-AVs
        # -> epilogue copies -> bc/recip/muls -> final outproj; the
        # reserved units (min_slot sentinel) are independent of all of
        # it, so emit them BETWEEN the AVs and the epilogue where the PE
        # would otherwise idle ~3us behind the DVE/ACT epilogue chain.
        reserved = [e for e in workq if e[1] >= 10 ** 8]
        rest = [e for e in workq if e[1] < 10 ** 8]
        last_epi = max(
            (i for i, e in enumerate(rest) if e[0] == EPI_NS), default=0
        )
        # 2 reserved units cover the epilogue-staging copies, the rest
        # cover the bc->recip->mul DVE chain (~2.5us) that the final
        # outproj waits on
        for _, _, u in rest[:last_epi + 1]:
            u()
        for _, _, u in reserved:
            u()
        for _, _, u in rest[last_epi + 1:]:
            u()

    for p in [small, work, p01, perm, const]:
        p.release()


_CACHE = {}


def _build():
    if "nc" in _CACHE:
        return _CACHE["nc"]
    nc = bacc.Bacc("TRN2", target_bir_lowering=False, debug=False, num_devices=8)
    # all inputs pre-arranged host-side into their SBUF tile layouts so
    # every DMA lands as 128 contiguous 4-8KB per-partition runs
    xT_d = nc.dram_tensor("xT", [4, 128, 4096], BF16, kind="ExternalInput").ap()
    wqT_d = nc.dram_tensor("wqT", [128, 8, E], BF16, kind="ExternalInput").ap()
    wkT_d = nc.dram_tensor("wkT", [128, 8, E], BF16, kind="ExternalInput").ap()
    wvT_d = nc.dram_tensor("wvT", [128, 8, E], BF16, kind="ExternalInput").ap()
    woT_d = nc.dram_tensor("woT", [128, 2, D], BF16, kind="ExternalInput").ap()
    yT_d = nc.dram_tensor("yT", [4, 128, 4096], BF16, kind="ExternalOutput").ap()
    mask_d = nc.dram_tensor("maskc", [128, 128], BF16, kind="ExternalInput").ap()
    ones_d = nc.dram_tensor("onesc", [128, 64], F32, kind="ExternalInput").ap()
    with tile.TileContext(nc) as tc:
        _emit(tc, nc, xT_d, wqT_d, wkT_d, wvT_d, woT_d, yT_d, mask_d, ones_d)
    nc.compile()
    _CACHE["nc"] = nc
    return nc


_r = np.arange(128)
_MASK = np.where(_r[:, None] <= _r[None, :], 1.0, 0.0).astype(ml_dtypes.bfloat16)
_ONES = np.ones((128, 64), dtype=np.float32)

LAST_RESULT = None


def kernel(x, wq, wk, wv, wo):
    global LAST_RESULT
    nc = _build()
    bf = ml_dtypes.bfloat16
    x = np.asarray(x, dtype=np.float32)
    wq = np.asarray(wq, dtype=np.float32)
    wk = np.asarray(wk, dtype=np.float32)
    wv = np.asarray(wv, dtype=np.float32)
    wo = np.asarray(wo, dtype=np.float32)

    def wlay(w2d):  # [256 out, 1024 in] -> [p, dc, e] = w2d[e, dc*128+p]
        return np.ascontiguousarray(
            w2d.T.reshape(8, 128, 256).transpose(1, 0, 2).astype(bf))

    in_maps = []
    for c in range(8):
        b, g = c // 4, c % 4
        rows = slice(g * E, (g + 1) * E)
        # xT dram [sc, p, dc*512+c] = x[sc*512+c, dc*128+p]
        xr = x[b].reshape(4, 512, 8, 128).transpose(0, 3, 2, 1).reshape(4, 128, 4096)
        # woT dram [p, ec, o] = wo[o, ec*128+p (within rows)]
        wor = wo[:, rows].T.reshape(2, 128, 1024).transpose(1, 0, 2)
        in_maps.append({
            "xT": np.ascontiguousarray(xr.astype(bf)),
            "wqT": wlay(wq[rows]),
            "wkT": wlay(wk[rows]),
            "wvT": wlay(wv[rows]),
            "woT": np.ascontiguousarray(wor.astype(bf)),
            "maskc": _MASK,
            "onesc": _ONES,
        })

    res = bass_utils.run_bass_kernel_spmd(nc, in_maps, core_ids=list(range(8)))
    LAST_RESULT = res

    y = np.empty((B, S, D), dtype=np.float32)
    for b in range(B):
        acc = res.results[4 * b]["yT"].astype(np.float32)
        for g in range(1, 4):
            acc += res.results[4 * b + g]["yT"].astype(np.float32)
        # yT dram [wi, p, dc*512+c] = y[wi*512+c, dc*128+p]
        y[b] = acc.reshape(4, 128, 8, 512).transpose(0, 3, 2, 1).reshape(S, D)
    return y

